# revision 1
# baseline (speedup 1.0000x reference)
"""Trainium2 Bass kernel for nn_MultiHeadAttention (B=2, S=2048, D=1024, H=16).

Sharding: 8 cores = 2 batches x 4 head-groups (4 heads / 256 dims each).
Each core computes its head-group's QKV projections, attention, and a
partial output projection (Megatron row-parallel); host sums the 4
partials per batch and adds the bias terms.

All operands are fp16: halves HBM traffic vs fp32 and runs every matmul
at 1 cycle/row regardless of moving-dim width. Attention uses a q-major
AV layout (out [q,dv] + separate denominator column); x[q,dv] is flipped
to x^T[dv,q] with DMA-engine xbar transposes.

Engines execute their queues in order, so emission order is the
schedule. The Activation engine's 128 exp tiles (~133us) are the window
floor. DMAs are priority-ordered so the first exp starts ~17us in; all
deferred work (m=1 projection chunks as 4-matmul half-chains, per-head
V projections, output-projection chunks, transposes) is hooked into the
attention kc-loops sized to each section's PE slack, so exp rarely
waits.
"""
import sys
sys.path.insert(0, '/opt/trn_rl_repo')

from contextlib import ExitStack

import numpy as np

import concourse.bass as bass
import concourse.mybir as mybir
import concourse.tile as tile
from concourse import bacc
from concourse.bass_utils import run_bass_kernel_spmd

B, S, D, H = 2, 2048, 1024, 16
HD = D // H            # 64
NCORES = 8
GROUPS = 4             # head groups (tensor parallel)
DL = D // GROUPS       # 256 local d_out per core
HL = H // GROUPS       # 4 local heads
P = 128
KC = S // P            # 16 k-chunks
SC = D // P            # 8 d_in chunks
F16 = mybir.dt.float16
F32 = mybir.dt.float32


def _build_module():
    nc = bacc.Bacc(None, target_bir_lowering=False, debug=False)

    qT = nc.dram_tensor("qT", [D, S], F16, kind="ExternalInput").ap()
    kT = nc.dram_tensor("kT", [D, S], F16, kind="ExternalInput").ap()
    vT = nc.dram_tensor("vT", [D, S], F16, kind="ExternalInput").ap()
    wqT = nc.dram_tensor("wqT", [D, DL], F16, kind="ExternalInput").ap()
    wkT = nc.dram_tensor("wkT", [D, DL], F16, kind="ExternalInput").ap()
    wvT = nc.dram_tensor("wvT", [D, DL], F16, kind="ExternalInput").ap()
    woT = nc.dram_tensor("woT", [DL, D], F16, kind="ExternalInput").ap()
    bq2 = nc.dram_tensor("bq2", [2, P], F32, kind="ExternalInput").ap()
    bk2 = nc.dram_tensor("bk2", [2, P], F32, kind="ExternalInput").ap()
    out = nc.dram_tensor("out", [S, D], F16, kind="ExternalOutput").ap()

    qTv = qT.rearrange("(kc p) s -> p kc s", p=P)
    kTv = kT.rearrange("(kc p) s -> p kc s", p=P)
    vTv = vT.rearrange("(kc p) s -> p kc s", p=P)
    outv = out.rearrange("(g c p) n -> g p c n", p=P, c=4)  # 4-row-chunk groups

    with tile.TileContext(nc) as tc:
        with ExitStack() as ctx:
            wpool = ctx.enter_context(tc.tile_pool(name="weights", bufs=1))
            big = ctx.enter_context(tc.tile_pool(name="big", bufs=1))
            qslab = ctx.enter_context(tc.tile_pool(name="qslab", bufs=4))
            kslab = ctx.enter_context(tc.tile_pool(name="kslab", bufs=4))
            vslab = ctx.enter_context(tc.tile_pool(name="vslab", bufs=4))
            ptp = ctx.enter_context(tc.tile_pool(name="pt", bufs=8))
            recp = ctx.enter_context(tc.tile_pool(name="rec", bufs=2))
            outp = ctx.enter_context(tc.tile_pool(name="outsb", bufs=1))

            # ---- persistent SBUF ----
            wq_sb = wpool.tile([P, SC, DL], F16)
            wk_sb = wpool.tile([P, SC, DL], F16)
            wv_sb = wpool.tile([P, SC, DL], F16)
            wo_sb = wpool.tile([P, DL // P, D], F16)
            bq_sb = wpool.tile([P, 2], F32)
            bk_sb = wpool.tile([P, 2], F32)
            QT = big.tile([P, 2, S], F16)           # [d_out in pair, m, q]
            KT = big.tile([P, 2, S], F16)
            V2 = big.tile([P, KC, HL, HD + 1], F16)  # [k, kc, head, V|one]
            xq0 = big.tile([P, KC, 2 * HD], F16)    # [q, qc, dv pair0]
            xq1 = big.tile([P, KC, 2 * HD], F16)
            xq = [xq0, xq1]
            xT = big.tile([P, 2, S], F16)           # [dv in pair, ci, q]
            osb_tiles = [outp.tile([P, 4, 1024], F16, tag="osb0", name="o0"),
                         outp.tile([P, 4, 1024], F16, tag="osb1", name="o1")]

            nc.gpsimd.memset(V2[:, :, :, HD:HD + 1], 1.0)

            # ---- PSUM: stA = score tiles (4 banks), stB = av+den (2),
            # stC = qk+pv (2, closed mid-stream for the out-proj pools) ----
            stA = ExitStack()
            st_ps = stA.enter_context(
                tc.tile_pool(name="st_ps", bufs=2, space="PSUM"))   # 4 banks
            stB = ExitStack()
            av_ps = stB.enter_context(
                tc.tile_pool(name="av_ps", bufs=1, space="PSUM"))   # 2 banks
            stC = ExitStack()
            qk_ps = stC.enter_context(
                tc.tile_pool(name="qk_ps", bufs=1, space="PSUM"))   # 1 bank
            pv_ps = stC.enter_context(
                tc.tile_pool(name="pv_ps", bufs=1, space="PSUM"))   # 1 bank

            # ---- emission helpers (emission order == engine order) ----
            slabs = {}
            qk_pending = {}

            def load(kind, view, j):
                pool = {"q": qslab, "k": kslab, "v": vslab}[kind]
                t = pool.tile([P, SC, 512], F16, tag=kind, name="slab_t")
                nc.sync.dma_start(t[:], view[:, :, j * 512:(j + 1) * 512])
                slabs[(kind, j)] = t

            def proj_qk(kind, j, m, lo=0, hi=SC):
                """Matmuls [lo, hi) of the 8-chunk accumulation chain for
                the m-chunk of Q/K projection, slab j. Small parts keep a
                hook's PE steal under ~0.5us so exp never waits."""
                t = slabs[(kind, j)]
                w_sb, b_sb, dst = ((wq_sb, bq_sb, QT) if kind == "q"
                                   else (wk_sb, bk_sb, KT))
                if lo == 0:
                    ps = qk_ps.tile([P, 512], F32, tag="qk", name="ps")
                    qk_pending[(kind, j, m)] = ps
                else:
                    ps = qk_pending[(kind, j, m)]
                for kc in range(lo, hi):
                    nc.tensor.matmul(
                        ps[:], w_sb[:, kc, m * P:(m + 1) * P], t[:, kc, :],
                        start=(kc == 0), stop=(kc == SC - 1))
                if hi == SC:
                    del qk_pending[(kind, j, m)]
                    nc.vector.tensor_scalar_add(
                        dst[:, m, j * 512:(j + 1) * 512], ps[:],
                        b_sb[:, m:m + 1])

            def proj_v(c, h):
                """V2[:, c, h, :]: head h's V columns for k-chunk c."""
                t = slabs[("v", c // 4)]
                ss = c % 4
                psv = pv_ps.tile([P, HD], F32, tag="pv", name="psv")
                for kc in range(SC):
                    nc.tensor.matmul(
                        psv[:], t[:, kc, ss * P:(ss + 1) * P],
                        wv_sb[:, kc, h * HD:(h + 1) * HD],
                        start=(kc == 0), stop=(kc == SC - 1))
                nc.vector.tensor_copy(V2[:, c, h, 0:HD], psv[:])

            op_pools = [None, None]   # mid-stream + tail out-proj pools

            def out_proj(j, dve_only=False):
                """Output-projection chunk for query rows [128j, ..+128)."""
                pool = op_pools[j % len(op_pools)]
                op = pool.tile([P, 1024], F32, tag="op", name="op")
                for n in range(2):
                    for ci in range(2):
                        nc.tensor.matmul(
                            op[:, n * 512:(n + 1) * 512],
                            xT[:, ci, j * P:(j + 1) * P],
                            wo_sb[:, ci, n * 512:(n + 1) * 512],
                            start=(ci == 0), stop=(ci == 1))
                g, c = j // 4, j % 4
                osb = osb_tiles[g % 2]
                if dve_only or j % 2 == 1:
                    nc.vector.tensor_copy(osb[:, c, :], op[:])
                else:
                    nc.scalar.copy(osb[:, c, :], op[:])
                if g == 3 and c in (1, 3):
                    # split the last group so the final DMA is short
                    hs = slice(c - 1, c + 1)
                    nc.sync.dma_start(outv[g][:, hs, :], osb[:, hs, :])
                elif g < 3 and c == 3:
                    nc.sync.dma_start(outv[g], osb[:])

            def transpose(pair, qc):
                nc.sync.dma_start_transpose(
                    xT[:, pair, qc * P:(qc + 1) * P], xq[pair][:, qc, :])

            def attention(h, qh, hooks, av_lag=1, carry_out=False):
                """One (head, q-half) pass; hooks[kc] emits deferred work.
                av_lag: how many kc the AV matmuls trail scores/exp — a
                deeper lag rides out late-arriving V slabs without the
                in-order PE queue stalling the exp feed. carry_out=True
                returns the unemitted tail (last AVs + normalize) as
                closures for the caller to hook into the next section."""
                hp, hm = (h % 2) * HD, h // 2
                pair = h // 2
                # two av banks: [q, qc 0-3 | 4-7, V dims + denominator]
                av_t = [av_ps.tile([P, 4, HD + 1], F32, tag="av_a",
                                   name="av_a"),
                        av_ps.tile([P, 4, HD + 1], F32, tag="av_b",
                                   name="av_b")]

                def emit_av(kc, pt):
                    # One start/stop per PSUM bank: start pends the whole
                    # 2KB zero region; later qc first-writes zero-fill it.
                    for qc in range(8):
                        nc.tensor.matmul(
                            av_t[qc // 4][:, qc % 4, :],
                            pt[:, qc * P:(qc + 1) * P],
                            V2[:, kc, h, :],
                            start=(kc == 0 and qc % 4 == 0),
                            stop=(kc == KC - 1 and qc % 4 == 3),
                            skip_group_check=True)

                def norm():
                    rec = recp.tile([P, 8], F32, tag="rec", name="rec")
                    for i in range(2):
                        nc.vector.reciprocal(
                            rec[:, i * 4:(i + 1) * 4], av_t[i][:, :, HD])
                        nc.vector.tensor_tensor(
                            xq[pair][:, qh * 8 + i * 4:qh * 8 + i * 4 + 4,
                                     hp:hp + HD],
                            av_t[i][:, :, 0:HD],
                            rec[:, i * 4:(i + 1) * 4, None].to_broadcast(
                                [P, 4, HD]),
                            mybir.AluOpType.mult)

                pending = []
                for kc in range(KC if carry_out else KC + av_lag):
                    for hook in hooks.get(kc, ()):
                        hook()
                    if kc < KC:
                        st = st_ps.tile([P, 1024], F32, tag="st", name="st")
                        for qq in range(2):
                            q0 = qh * 1024 + qq * 512
                            nc.tensor.matmul(
                                st[:, qq * 512:(qq + 1) * 512],
                                KT[hp:hp + HD, hm, kc * P:(kc + 1) * P],
                                QT[hp:hp + HD, hm, q0:q0 + 512],
                                start=True, stop=True)
                        pt = ptp.tile([P, 1024], F16, tag="pt", name="pt")
                        nc.scalar.activation(
                            pt[:], st[:],
                            mybir.ActivationFunctionType.Exp, scale=0.125)
                        pending.append((kc, pt))
                    if len(pending) > (av_lag if kc < KC else 0):
                        emit_av(*pending.pop(0))
                if carry_out:
                    carry = [lambda kc=kc, pt=pt: emit_av(kc, pt)
                             for kc, pt in pending]
                    carry.append(norm)
                    return carry
                while pending:
                    emit_av(*pending.pop(0))
                norm()

            # ---- prologue: DMAs in priority order, pipelined first projs --
            nc.sync.dma_start(wq_sb[:], wqT.rearrange("(kc p) m -> p kc m", p=P))
            nc.sync.dma_start(bq_sb[:], bq2.rearrange("m p -> p m"))
            nc.sync.dma_start(bk_sb[:], bk2.rearrange("m p -> p m"))
            def warm(n):
                """Dummy matmuls on wq to hold the PE p-state at full clock
                through the DMA-paced prologue (ramp needs 3us of
                continuous execution; any idle resets it)."""
                ps = qk_ps.tile([P, 512], F32, tag="qk", name="warm")
                for i in range(n):
                    nc.tensor.matmul(
                        ps[:, 0:DL], wq_sb[:, i % SC, 0:P],
                        wq_sb[:, (i + 1) % SC, :], start=True, stop=True)

            load("q", qTv, 0)
            warm(16)
            proj_qk("q", 0, 0)
            warm(8)
            load("q", qTv, 1)
            nc.sync.dma_start(wk_sb[:], wkT.rearrange("(kc p) m -> p kc m", p=P))
            load("k", kTv, 0)
            proj_qk("q", 1, 0)
            warm(8)
            proj_qk("k", 0, 0)
            load("k", kTv, 1)
            nc.sync.dma_start(wv_sb[:], wvT.rearrange("(kc p) m -> p kc m", p=P))
            load("v", vTv, 0)
            load("k", kTv, 2)
            load("v", vTv, 1)
            load("k", kTv, 3)
            load("v", vTv, 2)
            load("v", vTv, 3)
            nc.sync.dma_start(wo_sb[:], woT.rearrange("(c p) n -> p c n", p=P))
            load("q", qTv, 2)
            load("q", qTv, 3)

            def add_proj_hooks(hooks, kind, j, m, kc0):
                """Spread one projection chain over 4 hooks (2 matmuls
                each) at kc0..kc0+3 — each steals <0.5us of PE."""
                for ph in range(4):
                    hooks.setdefault(kc0 + ph, []).append(
                        lambda kind=kind, j=j, m=m, ph=ph:
                        proj_qk(kind, j, m, 2 * ph, 2 * ph + 2))

            def add_v_hooks(hooks, h, lag, cmax=KC):
                for c in range(cmax):
                    hooks.setdefault(c + lag, []).append(
                        lambda c=c: proj_v(c, h))

            # ---- attention sections (qh outer) with balanced hooks ----
            # sec0 (qh0,h0): k m0 full chains just-in-time (data-late, so
            # latency beats spreading) + h0's V chunks 0-11; the DMA-gated
            # tail (V 12-15, last 4 AVs, normalize) spills into sec1.
            hooks = {4: [lambda: proj_qk("k", 1, 0)],
                     5: [lambda: proj_qk("q", 0, 1)],
                     6: [lambda: proj_qk("q", 1, 1)],
                     8: [lambda: proj_qk("k", 2, 0)],
                     12: [lambda: proj_qk("k", 3, 0)]}
            add_v_hooks(hooks, 0, 1, cmax=12)
            carry = attention(0, 0, hooks, av_lag=4, carry_out=True)

            # sec1 (qh0,h1): sec0's spill packed early + h1's V + k0 m1
            # (needed by sec2 kc0) at the end.
            hooks = {0: [lambda: proj_v(12, 0), lambda: proj_v(13, 0),
                         carry[0], carry[1]],
                     1: [lambda: proj_v(14, 0), lambda: proj_v(15, 0),
                         carry[2], carry[3], carry[4]]}
            add_proj_hooks(hooks, "k", 0, 1, 9)
            add_v_hooks(hooks, 1, 1)
            attention(1, 0, hooks, av_lag=2)
            for qc in range(8):
                transpose(0, qc)

            # sec2 (qh0,h2): h2's V + k1/k2/k3 m1 ahead of their columns.
            hooks = {}
            add_proj_hooks(hooks, "k", 1, 1, 0)
            add_proj_hooks(hooks, "k", 2, 1, 4)
            add_proj_hooks(hooks, "k", 3, 1, 8)
            add_v_hooks(hooks, 2, 1)
            attention(2, 0, hooks)

            # sec3 (qh0,h3): h3's V + q2/q3 m0 (for qh1) + q2 m1.
            hooks = {}
            add_proj_hooks(hooks, "q", 2, 0, 2)
            add_proj_hooks(hooks, "q", 3, 0, 7)
            add_proj_hooks(hooks, "q", 2, 1, 11)
            add_v_hooks(hooks, 3, 1)
            attention(3, 0, hooks)
            for qc in range(8):
                transpose(1, qc)

            # sec4 (qh1,h0): last projection (q3 m1), then close the qk/pv
            # banks and open the mid-stream out-proj pool.
            def open_op_mid():
                # right-side stack: av (left) closes later, beneath it
                stC.close()
                op_pools[:] = [tc.alloc_tile_pool(
                    name="op_mid", bufs=1, space="PSUM", side="right")]
            hooks = {6: [open_op_mid],
                     8: [lambda: out_proj(0, True)],
                     12: [lambda: out_proj(1, True)]}
            add_proj_hooks(hooks, "q", 3, 1, 1)
            attention(0, 1, hooks)
            attention(1, 1, {4: [lambda: out_proj(2, True)],
                             12: [lambda: out_proj(3, True)]})
            for qc in range(8, 16):
                transpose(0, qc)

            attention(2, 1, {4: [lambda: out_proj(4, True)],
                             12: [lambda: out_proj(5, True)]})
            attention(3, 1, {4: [lambda: out_proj(6, True)],
                             12: [lambda: out_proj(7, True)]})

            # ---- tail: score + av banks are done, free them and rotate
            # out-proj through 3 pools; interleave the final transposes ----
            stB.close()
            stA.close()
            op_pools.append(tc.alloc_tile_pool(name="op_t1", bufs=1,
                                               space="PSUM", side="left"))
            op_pools.append(tc.alloc_tile_pool(name="op_t2", bufs=1,
                                               space="PSUM", side="left"))
            for j in range(8, 16):
                transpose(1, j)
                out_proj(j)
            op_pools[2].release()
            op_pools[1].release()
            op_pools[0].release()

    nc.compile()
    return nc


_NC = None


def _get_nc():
    global _NC
    if _NC is None:
        _NC = _build_module()
    return _NC


def kernel(query, key, value, mask, Wq, bq, Wk, bk, Wv, bv, Wo, bo,
           _trace=False):
    query = np.asarray(query, np.float32)
    key = np.asarray(key, np.float32)
    value = np.asarray(value, np.float32)
    Wq, Wk, Wv, Wo = (np.asarray(w, np.float32) for w in (Wq, Wk, Wv, Wo))
    bq, bk, bv, bo = (np.asarray(b_, np.float32) for b_ in (bq, bk, bv, bo))
    mask = np.asarray(mask, bool)

    f16 = lambda x: np.ascontiguousarray(x, np.float16)
    qT = [f16(query[b].T) for b in range(B)]
    kTh = [f16(key[b].T) for b in range(B)]
    vTh = [f16(value[b].T) for b in range(B)]

    in_maps = []
    for c in range(NCORES):
        b, g = c // GROUPS, c % GROUPS
        gs = slice(g * DL, (g + 1) * DL)
        in_maps.append({
            "qT": qT[b], "kT": kTh[b], "vT": vTh[b],
            "wqT": f16(Wq[gs, :].T),
            "wkT": f16(Wk[gs, :].T),
            "wvT": f16(Wv[gs, :].T),
            "woT": f16(Wo[:, gs].T),
            "bq2": np.ascontiguousarray(bq[gs].reshape(2, P)),
            "bk2": np.ascontiguousarray(bk[gs].reshape(2, P)),
        })

    nc = _get_nc()
    res = run_bass_kernel_spmd(nc, in_maps, core_ids=list(range(NCORES)),
                               trace=_trace)

    extra = (bv @ Wo.T + bo).astype(np.float32)  # bv folds through out-proj
    output = np.zeros((B, S, D), np.float32)
    for c in range(NCORES):
        output[c // GROUPS] += res.results[c]["out"].astype(np.float32)
    output += extra

    # masked query rows attend uniformly (softmax of constant -1e9)
    if mask.any():
        for b in range(B):
            rows = np.nonzero(mask[b, 0])[0]
            if rows.size:
                v_full = value[b] @ Wv.T + bv
                out_row = v_full.mean(0) @ Wo.T + bo
                output[b, rows, :] = out_row

    if _trace:
        return output, res
    return output

